# revision 15
# baseline (speedup 1.0000x reference)
"""MoE basic block kernel for 8 Trainium2 NeuronCores.

Strategy: data-parallel over batch (2 images per core). Training-mode
(sync) batchnorm stats are combined with a tiny cross-core AllReduce.
Convs run on the tensor engine as 18 accumulating matmuls per output
tile (2 input-channel tiles x 9 spatial taps) over a zero-padded input
image held in SBUF, in float32r (full-rate fp32 matmul mode).
"""

import sys

sys.path.insert(0, "/opt/trn_rl_repo")

import numpy as np

import concourse.bacc as bacc
import concourse.tile as tile
from concourse import mybir
from concourse.bass_utils import run_bass_kernel_spmd

P = 128
B, C, H, W, E = 16, 256, 64, 64, 512
NCORES = 8
BL = B // NCORES          # images per core
CT = C // P               # channel tiles
ET = E // P               # embedding tiles
RB = 8                    # rows per band
NB = H // RB              # bands per image
HP, WP = H + 2, W + 2     # padded image
EPS = 1e-5

F32 = mybir.dt.float32
F32R = mybir.dt.float32r


def build_nc():
    nc = bacc.Bacc()

    x_in = nc.dram_tensor("x", [BL, C, H, W], F32, kind="ExternalInput")
    embT = nc.dram_tensor("embT", [E, BL], F32, kind="ExternalInput")
    w1T = nc.dram_tensor("w1T", [9, C, C], F32, kind="ExternalInput")
    w2T = nc.dram_tensor("w2T", [9, C, C], F32, kind="ExternalInput")
    g1w = nc.dram_tensor("g1w", [E, C], F32, kind="ExternalInput")
    g2w = nc.dram_tensor("g2w", [E, C], F32, kind="ExternalInput")
    g1b = nc.dram_tensor("g1b", [C], F32, kind="ExternalInput")
    g2b = nc.dram_tensor("g2b", [C], F32, kind="ExternalInput")
    bn1g = nc.dram_tensor("bn1g", [C], F32, kind="ExternalInput")
    bn1b = nc.dram_tensor("bn1b", [C], F32, kind="ExternalInput")
    bn2g = nc.dram_tensor("bn2g", [C], F32, kind="ExternalInput")
    bn2b = nc.dram_tensor("bn2b", [C], F32, kind="ExternalInput")

    out_t = nc.dram_tensor("out", [BL, C, H, W], F32, kind="ExternalOutput")
    gate1_t = nc.dram_tensor("gate1", [BL, C], F32, kind="ExternalOutput")
    gate2_t = nc.dram_tensor("gate2", [BL, C], F32, kind="ExternalOutput")

    rg = [list(range(NCORES))]

    with tile.TileContext(nc) as tc:
        with (
            tc.tile_pool(name="pad_pool", bufs=1) as pad_pool,
            tc.tile_pool(name="w_pool", bufs=1) as w_pool,
            tc.tile_pool(name="y_pool", bufs=1) as y_pool,
            tc.tile_pool(name="small", bufs=1) as small,
            tc.tile_pool(name="xr_pool", bufs=3) as xr_pool,
            tc.tile_pool(name="wstg_pool", bufs=2) as wstg_pool,
            tc.tile_pool(name="st_pool", bufs=2) as st_pool,
            tc.tile_pool(name="ost_pool", bufs=2) as ost_pool,
            tc.tile_pool(name="cpsum", bufs=4, space="PSUM") as cpsum,
            tc.tile_pool(name="spsum", bufs=1, space="PSUM") as spsum,
            tc.tile_pool(name="dram", bufs=1, space="DRAM") as dram,
        ):
            # ---------------- persistent SBUF buffers ----------------
            pad = pad_pool.tile([P, CT, BL, HP, WP], F32R, tag="pad", name="x_pad")
            y_store = y_pool.tile([P, CT, BL, H, W], F32, name="y_store")

            # zero source for border writes (memset cannot target f32r;
            # ACT Copy rounds legally into the f32r pad buffer)
            zrow = pad_pool.tile([P, WP], F32, name="zrow")
            nc.vector.memset(zrow, 0.0)

            def zero_borders(t):
                cp = mybir.ActivationFunctionType.Copy
                for ct in range(CT):
                    for b in range(BL):
                        nc.scalar.activation(t[:, ct, b, 0, :], zrow, cp)
                        nc.scalar.activation(t[:, ct, b, HP - 1, :], zrow, cp)
                        nc.scalar.activation(
                            t[:, ct, b, 1 : HP - 1, 0], zrow[:, :H], cp
                        )
                        nc.scalar.activation(
                            t[:, ct, b, 1 : HP - 1, WP - 1], zrow[:, :H], cp
                        )

            zero_borders(pad)

            # input image load into padded layout: DMA to a small staging
            # tile, then ACT copy-cast into the float32r padded buffer
            # (fp32r matmul operands must be written by a rounding engine op).
            for ct in range(CT):
                for b in range(BL):
                    for band in range(NB):
                        xs = xr_pool.tile([P, RB, W], F32, tag="xstage", name="xs")
                        nc.sync.dma_start(
                            out=xs,
                            in_=x_in[
                                b,
                                ct * P : (ct + 1) * P,
                                band * RB : (band + 1) * RB,
                                :,
                            ],
                        )
                        nc.scalar.activation(
                            pad[
                                :,
                                ct,
                                b,
                                1 + band * RB : 1 + (band + 1) * RB,
                                1 : 1 + W,
                            ],
                            xs,
                            mybir.ActivationFunctionType.Copy,
                        )

            # conv1 weights: [kk, ci, co] -> per ci-tile [ci_p, kk, co],
            # via fp32 staging + DVE cast to float32r
            def load_weights(wT, name):
                w_sb = w_pool.tile([P, CT, 9, C], F32R, tag="w", name=name)
                for ci_t in range(CT):
                    ws = wstg_pool.tile([P, 9, C], F32, tag="wstg", name="wstg")
                    nc.sync.dma_start(
                        out=ws,
                        in_=wT[:, ci_t * P : (ci_t + 1) * P, :].rearrange(
                            "kk ci co -> ci kk co"
                        ),
                    )
                    nc.vector.tensor_copy(w_sb[:, ci_t], ws)
                return w_sb

            w1_sb = load_weights(w1T, "w1_sb")

            # ---------------- small constants ----------------
            embT_sb = small.tile([P, ET, BL], F32, name="embT_sb")
            nc.sync.dma_start(
                out=embT_sb, in_=embT.rearrange("(et p) b -> p et b", p=P)
            )
            cvecs = {}
            for name, t in (
                ("g1b", g1b),
                ("g2b", g2b),
                ("bn1g", bn1g),
                ("bn1b", bn1b),
                ("bn2g", bn2g),
                ("bn2b", bn2b),
            ):
                v = small.tile([P, CT], F32, name=f"{name}_sb")
                nc.sync.dma_start(out=v, in_=t.rearrange("(ct p) -> p ct", p=P))
                cvecs[name] = v

            ones_sb = small.tile([P, P], F32, name="ones_sb")
            nc.vector.memset(ones_sb, 1.0)
            eps_sb = small.tile([P, 1], F32, name="eps_sb")
            nc.vector.memset(eps_sb, EPS)

            # ---------------- gates (layout [c, b], softmax over c) ---------
            def compute_gate(idx, gw_dram, gb_sb, gate_out):
                gw_sb = small.tile([P, ET, C], F32, tag="gw", name=f"gw{idx}_sb")
                nc.sync.dma_start(
                    out=gw_sb, in_=gw_dram.rearrange("(et p) c -> p et c", p=P)
                )
                e_sb = small.tile([P, CT, BL], F32, name=f"e{idx}_sb")
                for ct in range(CT):
                    ps = spsum.tile([P, BL], F32, name="gate_ps")
                    for et in range(ET):
                        nc.tensor.matmul(
                            ps,
                            gw_sb[:, et, ct * P : (ct + 1) * P],
                            embT_sb[:, et, :],
                            start=(et == 0),
                            stop=(et == ET - 1),
                        )
                    # e = exp(u + bias)
                    nc.scalar.activation(
                        e_sb[:, ct, :],
                        ps,
                        mybir.ActivationFunctionType.Exp,
                        bias=gb_sb[:, ct : ct + 1],
                    )
                # sum over all channels, broadcast to all partitions
                ssum = spsum.tile([P, BL], F32, name="gate_ssum")
                for ct in range(CT):
                    nc.tensor.matmul(
                        ssum,
                        ones_sb,
                        e_sb[:, ct, :],
                        start=(ct == 0),
                        stop=(ct == CT - 1),
                    )
                rec = small.tile([P, BL], F32, name=f"rec{idx}_sb")
                nc.vector.reciprocal(rec, ssum)
                gate_cb = small.tile([P, CT, BL], F32, name=f"gate{idx}_cb")
                for ct in range(CT):
                    nc.vector.tensor_mul(gate_cb[:, ct, :], e_sb[:, ct, :], rec)
                    nc.sync.dma_start(
                        out=gate_out[:, ct * P : (ct + 1) * P].rearrange(
                            "b c -> c b"
                        ),
                        in_=gate_cb[:, ct, :],
                    )
                return gate_cb

            gate1_cb = compute_gate(1, g1w, cvecs["g1b"], gate1_t)
            gate2_cb = compute_gate(2, g2w, cvecs["g2b"], gate2_t)

            # ---------------- conv + gate + stats ----------------
            def conv(idx, src_pad, w_sb, gate_cb):
                stats = small.tile(
                    [P, CT, BL * NB, 6], F32, name=f"stats{idx}"
                )
                for co_t in range(CT):
                    for b in range(BL):
                        for band in range(NB):
                            ps = cpsum.tile([P, RB, W], F32, name="conv_ps", tag="conv_ps")
                            mm = 0
                            for ci_t in range(CT):
                                for kk in range(9):
                                    ky, kx = divmod(kk, 3)
                                    nc.tensor.matmul(
                                        ps,
                                        w_sb[:, ci_t, kk, co_t * P : (co_t + 1) * P],
                                        src_pad[
                                            :,
                                            ci_t,
                                            b,
                                            band * RB + ky : band * RB + ky + RB,
                                            kx : kx + W,
                                        ],
                                        start=(mm == 0),
                                        stop=(mm == CT * 9 - 1),
                                    )
                                    mm += 1
                            yt = y_store[:, co_t, b, band * RB : (band + 1) * RB, :]
                            nc.vector.tensor_scalar_mul(
                                yt, ps, gate_cb[:, co_t, b : b + 1]
                            )
                            nc.vector.bn_stats(
                                stats[:, co_t, b * NB + band, :],
                                yt.rearrange("p h w -> p (h w)"),
                            )
                # local stats -> (mean/8, E[x^2]/8) per channel
                mv = small.tile([P, CT, 2], F32, name=f"mv{idx}")
                loc = small.tile([P, CT, 2], F32, name=f"loc{idx}")
                for ct in range(CT):
                    nc.vector.bn_aggr(mv[:, ct], stats[:, ct])
                    nc.vector.tensor_mul(
                        loc[:, ct, 1:2], mv[:, ct, 0:1], mv[:, ct, 0:1]
                    )
                    nc.vector.tensor_add(
                        loc[:, ct, 1:2], loc[:, ct, 1:2], mv[:, ct, 1:2]
                    )
                    nc.vector.tensor_scalar_mul(
                        loc[:, ct, 1:2], loc[:, ct, 1:2], 1.0 / NCORES
                    )
                    nc.vector.tensor_scalar_mul(
                        loc[:, ct, 0:1], mv[:, ct, 0:1], 1.0 / NCORES
                    )
                # cross-core allreduce of the 2*CT per-channel scalars
                ar_in = dram.tile([P, CT * 2], F32, name=f"ar{idx}_in")
                ar_out = dram.tile(
                    [P, CT * 2], F32, addr_space="Shared", name=f"ar{idx}_out"
                )
                nc.sync.dma_start(
                    out=ar_in, in_=loc.rearrange("p ct two -> p (ct two)")
                )
                nc.gpsimd.collective_compute(
                    "AllReduce",
                    mybir.AluOpType.add,
                    ins=[ar_in[:, :]],
                    outs=[ar_out[:, :]],
                    replica_groups=rg,
                )
                glob = small.tile([P, CT, 2], F32, name=f"glob{idx}")
                nc.sync.dma_start(
                    out=glob.rearrange("p ct two -> p (ct two)"), in_=ar_out[:, :]
                )
                # scale = gamma * rsqrt(var+eps); shift = beta - mean*scale
                var_t = small.tile([P, CT], F32, name=f"var{idx}")
                sd = small.tile([P, CT], F32, name=f"sd{idx}")
                inv = small.tile([P, CT], F32, name=f"inv{idx}")
                scale = small.tile([P, CT], F32, name=f"scale{idx}")
                shift = small.tile([P, CT], F32, name=f"shift{idx}")
                for ct in range(CT):
                    nc.vector.tensor_mul(
                        var_t[:, ct : ct + 1], glob[:, ct, 0:1], glob[:, ct, 0:1]
                    )
                    nc.vector.tensor_tensor(
                        var_t[:, ct : ct + 1],
                        glob[:, ct, 1:2],
                        var_t[:, ct : ct + 1],
                        mybir.AluOpType.subtract,
                    )
                nc.scalar.activation(
                    sd, var_t, mybir.ActivationFunctionType.Sqrt, bias=eps_sb[:, 0:1]
                )
                nc.vector.reciprocal(inv, sd)
                gname = f"bn{idx}g"
                bname = f"bn{idx}b"
                nc.vector.tensor_mul(scale, cvecs[gname], inv)
                for ct in range(CT):
                    nc.vector.tensor_mul(
                        shift[:, ct : ct + 1], glob[:, ct, 0:1], scale[:, ct : ct + 1]
                    )
                nc.vector.tensor_tensor(
                    shift, cvecs[bname], shift, mybir.AluOpType.subtract
                )
                return scale, shift

            scale1, shift1 = conv(1, pad, w1_sb, gate1_cb)

            # ---------------- bn1 + relu -> padded conv2 input ----------
            z_pad = pad_pool.tile([P, CT, BL, HP, WP], F32R, tag="pad", name="z_pad")
            zero_borders(z_pad)
            for ct in range(CT):
                for b in range(BL):
                    for band in range(NB):
                        nc.scalar.activation(
                            z_pad[
                                :,
                                ct,
                                b,
                                1 + band * RB : 1 + (band + 1) * RB,
                                1 : 1 + W,
                            ],
                            y_store[:, ct, b, band * RB : (band + 1) * RB, :],
                            mybir.ActivationFunctionType.Relu,
                            bias=shift1[:, ct : ct + 1],
                            scale=scale1[:, ct : ct + 1],
                        )

            w2_sb = load_weights(w2T, "w2_sb")
            scale2, shift2 = conv(2, z_pad, w2_sb, gate2_cb)

            # ---------------- bn2 + residual + relu -> out ----------
            for co_t in range(CT):
                for b in range(BL):
                    for band in range(NB):
                        yt = y_store[:, co_t, b, band * RB : (band + 1) * RB, :]
                        xr = xr_pool.tile([P, RB, W], F32, tag="xstage", name="xr")
                        nc.sync.dma_start(
                            out=xr,
                            in_=x_in[
                                b,
                                co_t * P : (co_t + 1) * P,
                                band * RB : (band + 1) * RB,
                                :,
                            ],
                        )
                        st = st_pool.tile([P, RB, W], F32, name="st")
                        nc.vector.tensor_scalar(
                            st,
                            yt,
                            scale2[:, co_t : co_t + 1],
                            shift2[:, co_t : co_t + 1],
                            mybir.AluOpType.mult,
                            mybir.AluOpType.add,
                        )
                        nc.vector.tensor_add(st, st, xr)
                        ost = ost_pool.tile([P, RB, W], F32, name="ost")
                        nc.scalar.activation(
                            ost, st, mybir.ActivationFunctionType.Relu
                        )
                        nc.sync.dma_start(
                            out=out_t[
                                b,
                                co_t * P : (co_t + 1) * P,
                                band * RB : (band + 1) * RB,
                                :,
                            ],
                            in_=ost,
                        )

    nc.compile()
    return nc


def make_in_maps(x, embedding, conv1_w, conv2_w, g1_w, g1_b, g2_w, g2_b,
                 bn1_g, bn1_b, bn2_g, bn2_b):
    """Host-side layout-only pre-packing + batch sharding."""
    w1T = np.ascontiguousarray(
        np.asarray(conv1_w, dtype=np.float32).transpose(2, 3, 1, 0)
    ).reshape(9, C, C)
    w2T = np.ascontiguousarray(
        np.asarray(conv2_w, dtype=np.float32).transpose(2, 3, 1, 0)
    ).reshape(9, C, C)
    x = np.asarray(x, dtype=np.float32)
    embedding = np.asarray(embedding, dtype=np.float32)
    shared = {
        "w1T": w1T,
        "w2T": w2T,
        "g1w": np.asarray(g1_w, dtype=np.float32),
        "g2w": np.asarray(g2_w, dtype=np.float32),
        "g1b": np.asarray(g1_b, dtype=np.float32),
        "g2b": np.asarray(g2_b, dtype=np.float32),
        "bn1g": np.asarray(bn1_g, dtype=np.float32),
        "bn1b": np.asarray(bn1_b, dtype=np.float32),
        "bn2g": np.asarray(bn2_g, dtype=np.float32),
        "bn2b": np.asarray(bn2_b, dtype=np.float32),
    }
    in_maps = []
    for c in range(NCORES):
        sl = slice(c * BL, (c + 1) * BL)
        in_maps.append(
            {
                "x": np.ascontiguousarray(x[sl]),
                "embT": np.ascontiguousarray(embedding[sl].T),
                **shared,
            }
        )
    return in_maps


def assemble(results):
    out = np.concatenate([results[c]["out"] for c in range(NCORES)], axis=0)
    gate1 = np.concatenate([results[c]["gate1"] for c in range(NCORES)], axis=0)
    gate2 = np.concatenate([results[c]["gate2"] for c in range(NCORES)], axis=0)
    return out, gate1, gate2


def kernel(**inputs):
    in_maps = make_in_maps(**inputs)
    nc = build_nc()
    r = run_bass_kernel_spmd(nc, in_maps, list(range(NCORES)))
    return assemble(r.results)


if __name__ == "__main__":
    rng = np.random.default_rng(0)
    inputs = {
        "x": rng.standard_normal((B, C, H, W), dtype=np.float32),
        "embedding": rng.standard_normal((B, E), dtype=np.float32),
        "conv1_w": rng.standard_normal((C, C, 3, 3), dtype=np.float32) / 48,
        "conv2_w": rng.standard_normal((C, C, 3, 3), dtype=np.float32) / 48,
        "g1_w": rng.standard_normal((E, C), dtype=np.float32) / 23,
        "g1_b": np.zeros(C, np.float32),
        "g2_w": rng.standard_normal((E, C), dtype=np.float32) / 23,
        "g2_b": np.zeros(C, np.float32),
        "bn1_g": np.ones(C, np.float32),
        "bn1_b": np.zeros(C, np.float32),
        "bn2_g": np.ones(C, np.float32),
        "bn2_b": np.zeros(C, np.float32),
    }
    out, g1, g2 = kernel(**inputs)
    print(out.shape, g1.shape, g2.shape, out.dtype)
